# revision 1
# baseline (speedup 1.0000x reference)
"""AttentionPooling (segment softmax-pool) Trainium2 kernel, 8-way data parallel.

Math: s = x@W + b (per node); g = softmax(s) over all N; then per-segment
softmax of g pools x:  pooled[seg] = sum_i x_i * exp(g_i) / sum_j exp(g_j)
(the per-segment max-shift in the reference cancels exactly).

Sharding: nodes are split across 8 cores at segment boundaries (batch_idx is
sorted), so every segment lives on exactly one core.  Each core streams its x
shard twice: pass 1 computes s via a fused multiply+reduce on the vector
engine; a pair of tiny AllReduces produce the global softmax max/denominator;
pass 2 builds, per 128-node tile, a one-hot(node->segment-within-chunk)
matrix scaled by e_i = exp(g_i) on the vector engine and matmul-accumulates
onehot_e.T @ [x | 1] into PSUM per <=128-segment chunk.  Column 256 of the
accumulator is the per-segment denominator; one reciprocal+scale per chunk
finishes the job.  No gather/scatter is needed anywhere.
"""

import math
from contextlib import ExitStack

import numpy as np

import concourse.bass as bass
import concourse.bass_isa as bass_isa
import concourse.tile as tile
from concourse import bacc, mybir, bass_utils

P = 128
D = 256
F = D + 1  # matmul columns: x plus a trailing ones column (denominator)
XCOLS = D + 2  # x layout adds one more column carrying b (or the pad mask)
NCORES = 8
NSEG = 4096
NEG_BIG = -1.0e30
SENTINEL = 500.0  # idx offset for padding rows; outside [0, 128)

_prog_cache = {}

# Set by a driving harness to capture an NTFF profile of the run; the
# measured kernel time lands in LAST_EXEC_NS.
TRACE = False
LAST_EXEC_NS = None


def _snap(bounds, tgt, lo, hi):
    """Segment boundary nearest to node index tgt, clamped to (lo, hi)."""
    s = int(np.searchsorted(bounds, tgt))
    if s > 0 and abs(int(bounds[s - 1]) - tgt) < abs(int(bounds[s]) - tgt):
        s -= 1
    return max(lo, min(s, hi))


def _plan(batch_idx):
    N = batch_idx.shape[0]
    counts = np.bincount(batch_idx, minlength=NSEG)
    bounds = np.concatenate([[0], np.cumsum(counts)]).astype(np.int64)

    core_seg = [0]
    for c in range(1, NCORES):
        s = _snap(bounds, N * c // NCORES, core_seg[-1] + 1, NSEG - (NCORES - c))
        core_seg.append(s)
    core_seg.append(NSEG)

    C = 5
    chunk_seg = []
    for c in range(NCORES):
        s0c, s1c = core_seg[c], core_seg[c + 1]
        n0c, n1c = int(bounds[s0c]), int(bounds[s1c])
        ks = [s0c]
        for k in range(1, C):
            s = _snap(bounds, n0c + (n1c - n0c) * k // C, ks[-1] + 1, s1c - (C - k))
            ks.append(s)
        ks.append(s1c)
        segs = list(zip(ks[:-1], ks[1:]))
        for a, b2 in segs:
            assert 0 < b2 - a <= P, f"chunk with {b2 - a} segments"
        chunk_seg.append(segs)

    Tc = []
    for k in range(C):
        mx = 0
        for c in range(NCORES):
            a, b2 = chunk_seg[c][k]
            mx = max(mx, math.ceil(int(bounds[b2] - bounds[a]) / P))
        Tc.append(mx)
    return core_seg, chunk_seg, C, Tc, bounds


def _build_core_inputs(x, batch_idx, W, b, chunk_segs, bounds, C, Tc, T):
    bval = float(b[0])
    xp = np.zeros((T * P, XCOLS), dtype=np.float32)
    xp[:, D] = 1.0        # ones column -> per-segment denominator
    xp[:, D + 1] = NEG_BIG  # bias column: b for real rows, -1e30 for padding
    idxoff = np.full((T * P,), SENTINEL, dtype=np.float32)
    base = 0
    for k in range(C):
        a, b2 = chunk_segs[k]
        m0, m1 = int(bounds[a]), int(bounds[b2])
        L = m1 - m0
        r0 = base * P
        xp[r0:r0 + L, :D] = x[m0:m1]
        xp[r0:r0 + L, D + 1] = bval
        idxoff[r0:r0 + L] = (batch_idx[m0:m1] - a).astype(np.float32)
        base += Tc[k]
    idxT = np.ascontiguousarray(idxoff.reshape(T, P).T)
    return {"x": xp, "idxT": idxT}


def _make_wrep(W):
    wrep = np.zeros((P, XCOLS), dtype=np.float32)
    wrep[:, :D] = np.broadcast_to(W[:, 0], (P, D))
    wrep[:, D + 1] = 1.0
    return wrep


def _build_program(C, Tc):
    T = sum(Tc)
    f32 = mybir.dt.float32
    Alu = mybir.AluOpType
    Act = mybir.ActivationFunctionType

    nc = bacc.Bacc("TRN2", target_bir_lowering=False, debug=False,
                   num_devices=NCORES)
    x = nc.dram_tensor("x", [T * P, XCOLS], f32, kind="ExternalInput").ap()
    idxT = nc.dram_tensor("idxT", [P, T], f32, kind="ExternalInput").ap()
    wrep = nc.dram_tensor("wrep", [P, XCOLS], f32, kind="ExternalInput").ap()
    out = nc.dram_tensor("out", [C * P, D], f32, kind="ExternalOutput").ap()
    cc_max_in = nc.dram_tensor("cc_max_in", [1, 1], f32)
    cc_max_out = nc.dram_tensor("cc_max_out", [1, 1], f32, addr_space="Shared")
    cc_sum_in = nc.dram_tensor("cc_sum_in", [1, 1], f32)
    cc_sum_out = nc.dram_tensor("cc_sum_out", [1, 1], f32, addr_space="Shared")
    groups = [list(range(NCORES))]

    with tile.TileContext(nc) as tc, ExitStack() as ctx:
        const = ctx.enter_context(tc.tile_pool(name="const", bufs=1))
        idxT_sb = const.tile([P, T], f32, tag="idxT")
        wrep_sb = const.tile([P, XCOLS], f32, tag="wrep")
        rowb_i = const.tile([P, P], mybir.dt.int32, tag="rowbi")
        rowb = const.tile([P, P], f32, tag="rowb")
        s_all = const.tile([P, T], f32, tag="s_all")
        et = const.tile([P, T], f32, tag="et")
        e_all = const.tile([P, T], f32, tag="e_all")
        smax = const.tile([P, 1], f32, tag="smax")
        zcol = const.tile([P, 1], f32, tag="zcol")
        lmax = const.tile([P, 1], f32, tag="lmax")
        gmax = const.tile([1, 1], f32, tag="gmax")
        negm = const.tile([1, 1], f32, tag="negm")
        lz = const.tile([P, 1], f32, tag="lz")
        gz = const.tile([1, 1], f32, tag="gz")
        invz = const.tile([1, 1], f32, tag="invz")
        negm_col = const.tile([P, 1], f32, tag="negmcol")
        invz_col = const.tile([P, 1], f32, tag="invzcol")

        nc.sync.dma_start(idxT_sb[:], idxT[:, :])
        nc.sync.dma_start(wrep_sb[:], wrep[:, :])
        nc.gpsimd.iota(rowb_i[:], pattern=[[1, P]], base=0, channel_multiplier=0)
        nc.vector.tensor_copy(rowb[:], rowb_i[:])

        # ---- pass 1: s = x @ W + b (masked via bias column) ----
        xpool1 = ctx.enter_context(tc.tile_pool(name="x1", bufs=12))
        prodpool = ctx.enter_context(tc.tile_pool(name="prod", bufs=4))
        for t in range(T):
            xt = xpool1.tile([P, XCOLS], f32, tag="xt")
            nc.sync.dma_start(xt[:], x[t * P:(t + 1) * P, :])
            pr = prodpool.tile([P, XCOLS], f32, tag="pr")
            nc.vector.tensor_tensor(out=pr[:], in0=xt[:], in1=wrep_sb[:],
                                    op=Alu.mult)
            nc.scalar.activation(pr[:], pr[:], Act.Identity,
                                 accum_out=s_all[:, t:t + 1])

        # ---- global softmax stats ----
        nc.vector.reduce_max(smax[:], s_all[:], axis=mybir.AxisListType.X)
        nc.gpsimd.partition_all_reduce(lmax[:], smax[:], channels=P,
                                       reduce_op=bass_isa.ReduceOp.max)
        nc.sync.dma_start(cc_max_in[:, :], lmax[0:1, 0:1])
        nc.gpsimd.collective_compute(
            "AllReduce", Alu.max, replica_groups=groups,
            ins=[cc_max_in[:, :]], outs=[cc_max_out[:, :]])
        nc.sync.dma_start(gmax[:], cc_max_out[:, :])
        nc.vector.tensor_scalar_mul(negm[:], gmax[:], -1.0)
        nc.gpsimd.partition_broadcast(negm_col[:], negm[:])
        nc.scalar.activation(et[:], s_all[:], Act.Exp, bias=negm_col[:],
                             accum_out=zcol[:])
        nc.gpsimd.partition_all_reduce(lz[:], zcol[:], channels=P,
                                       reduce_op=bass_isa.ReduceOp.add)
        nc.sync.dma_start(cc_sum_in[:, :], lz[0:1, 0:1])
        nc.gpsimd.collective_compute(
            "AllReduce", Alu.add, replica_groups=groups,
            ins=[cc_sum_in[:, :]], outs=[cc_sum_out[:, :]])
        nc.sync.dma_start(gz[:], cc_sum_out[:, :])
        nc.vector.reciprocal(invz[:], gz[:])
        nc.gpsimd.partition_broadcast(invz_col[:], invz[:])
        # e = exp(g), g = exp(s - M) / Z
        nc.scalar.activation(e_all[:], et[:], Act.Exp, scale=invz_col[:])

        # ---- pass 2: per-chunk segment-sum via one-hot matmul ----
        xpool3 = ctx.enter_context(tc.tile_pool(name="x3", bufs=12))
        ohpool = ctx.enter_context(tc.tile_pool(name="oh", bufs=8))
        psumpool = ctx.enter_context(
            tc.tile_pool(name="psum", bufs=2, space="PSUM"))
        outpool = ctx.enter_context(tc.tile_pool(name="osb", bufs=2))
        dpool = ctx.enter_context(tc.tile_pool(name="dp", bufs=2))
        tbase = 0
        for k in range(C):
            ps = psumpool.tile([P, F], f32, tag="ps")
            for j in range(Tc[k]):
                t = tbase + j
                xt = xpool3.tile([P, XCOLS], f32, tag="x3")
                nc.sync.dma_start(xt[:], x[t * P:(t + 1) * P, :])
                oh = ohpool.tile([P, P], f32, tag="oh")
                nc.vector.tensor_scalar(
                    out=oh[:], in0=rowb[:], scalar1=idxT_sb[:, t:t + 1],
                    scalar2=e_all[:, t:t + 1], op0=Alu.is_equal, op1=Alu.mult)
                nc.tensor.matmul(ps[:], lhsT=oh[:], rhs=xt[:, :F],
                                 start=(j == 0), stop=(j == Tc[k] - 1))
            den = dpool.tile([P, 1], f32, tag="den")
            nc.vector.tensor_scalar_max(den[:], ps[:, D:D + 1], 0.5)
            rec = dpool.tile([P, 1], f32, tag="rec")
            nc.vector.reciprocal(rec[:], den[:])
            osb = outpool.tile([P, D], f32, tag="osb")
            nc.vector.tensor_scalar(out=osb[:], in0=ps[:, :D],
                                    scalar1=rec[:], scalar2=None, op0=Alu.mult)
            nc.sync.dma_start(out[k * P:(k + 1) * P, :], osb[:])
            tbase += Tc[k]

    nc.compile()
    return nc


def _get_program(C, Tc):
    key = (C, tuple(Tc))
    if key not in _prog_cache:
        _prog_cache[key] = _build_program(C, Tc)
    return _prog_cache[key]


def kernel(x, batch_idx, W, b, num_segments):
    x = np.asarray(x, dtype=np.float32)
    batch_idx = np.asarray(batch_idx)
    W = np.asarray(W, dtype=np.float32)
    b = np.asarray(b, dtype=np.float32)
    assert int(num_segments) == NSEG and x.shape[1] == D

    core_seg, chunk_seg, C, Tc, bounds = _plan(batch_idx)
    T = sum(Tc)
    nc = _get_program(C, Tc)

    wrep = _make_wrep(W)
    in_maps = []
    for c in range(NCORES):
        m = _build_core_inputs(x, batch_idx, W, b, chunk_seg[c], bounds, C, Tc, T)
        m["wrep"] = wrep
        in_maps.append(m)

    global LAST_EXEC_NS
    res = bass_utils.run_bass_kernel_spmd(
        nc, in_maps, core_ids=list(range(NCORES)), trace=TRACE)
    if res.exec_time_ns is not None:
        LAST_EXEC_NS = res.exec_time_ns

    full = np.zeros((NSEG, D), dtype=np.float32)
    for c in range(NCORES):
        oc = res.results[c]["out"]
        for k in range(C):
            a, b2 = chunk_seg[c][k]
            full[a:b2] = oc[k * P:k * P + (b2 - a)]
    return full



# revision 2
# speedup vs baseline: 1.0043x; 1.0043x over previous
"""AttentionPooling (segment softmax-pool) Trainium2 kernel, 8-way data parallel.

Math: s = x@W + b; g = softmax(s) over all N; pooled[seg] = softmax-weighted
sum of x with weights softmax_seg(g).  The bias b cancels exactly in g, and
the per-segment max-shift cancels in the final softmax, so
  w_i  =  exp(g_i) / sum_seg exp(g_j),   g_i = exp(s_i) / Z0,  Z0 = sum exp(s).
Since g_i is tiny here, exp(g_i) = 1 + g_i to ~1e-9: the pooled output is
  (A0 + a*A1) / (c0 + a*c1),  a = 1/Z0,
  A0 = segsum(x), A1 = segsum(u*x), c0 = counts, c1 = segsum(u), u = exp(s).
All four accumulate in ONE matmul per 128-node tile via a stacked one-hot
lhsT: columns 0..SPAN-1 hold onehot(seg), columns SPAN..2*SPAN-1 onehot*u.
A single 1-scalar AllReduce produces Z0; a grouped combine finishes.

Single pass over x (fp16-packed on host), G tiles per DMA.  s is computed
on-device: packed multiply by W then a binary-tree reduction on the vector
engine (all operands packed-last for the 4x DVE mode); exp per group on the
scalar engine.  One-hot builds use a transposed [P, 2*SPAN, G] layout so
every operand keeps a packed last dim; the matmul reads strided lhsT slices.
"""

from contextlib import ExitStack

import numpy as np

import concourse.bass_isa as bass_isa
import concourse.tile as tile
from concourse import bacc, mybir, bass_utils

P = 128
D = 256
XC = D + 2          # x, ones column, even-stride pad column
RC = D + 1          # matmul rhs columns (x + ones)
NCORES = 8
NSEG = 4096
SPAN = 32           # max segments per chunk (stacked one-hot: 2*SPAN lhsT cols)
G = 32              # tiles per DMA/compute group
SENT = 500.0        # idx value for padding rows; never matches 0..SPAN-1

f16 = mybir.dt.float16
f32 = mybir.dt.float32

_prog_cache = {}

TRACE = False
LAST_EXEC_NS = None


# ---------------------------------------------------------------- host plan
def _snap(bounds, tgt, lo, hi):
    s = int(np.searchsorted(bounds, tgt))
    if s > 0 and abs(int(bounds[s - 1]) - tgt) < abs(int(bounds[s]) - tgt):
        s -= 1
    return max(lo, min(s, hi))


def _plan(batch_idx):
    N = batch_idx.shape[0]
    counts = np.bincount(batch_idx, minlength=NSEG)
    bounds = np.concatenate([[0], np.cumsum(counts)]).astype(np.int64)

    core_seg = [0]
    for c in range(1, NCORES):
        s = _snap(bounds, N * c // NCORES, core_seg[-1] + 1, NSEG - (NCORES - c))
        core_seg.append(s)
    core_seg.append(NSEG)

    C = max(-(-(core_seg[c + 1] - core_seg[c]) // SPAN) for c in range(NCORES))

    chunk_seg = []
    for c in range(NCORES):
        s0c, s1c = core_seg[c], core_seg[c + 1]
        n0c, n1c = int(bounds[s0c]), int(bounds[s1c])
        ks = [s0c]
        for k in range(1, C):
            lo = max(ks[-1] + 1, s1c - SPAN * (C - k))
            hi = min(ks[-1] + SPAN, s1c - (C - k) + 1)
            s = _snap(bounds, n0c + (n1c - n0c) * k // C, lo, hi)
            ks.append(s)
        ks.append(s1c)
        segs = list(zip(ks[:-1], ks[1:]))
        for a, b2 in segs:
            assert 0 <= b2 - a <= SPAN, f"chunk with {b2 - a} segments"
        chunk_seg.append(segs)

    Tc = []
    for k in range(C):
        mx = 1
        for c in range(NCORES):
            a, b2 = chunk_seg[c][k]
            mx = max(mx, -(-int(bounds[b2] - bounds[a]) // P))
        Tc.append(mx)
    return core_seg, chunk_seg, C, Tc, bounds


def _build_core_inputs(x16, batch_idx, padrow, chunk_segs, bounds, C, Tc, Tpad):
    """Pack one core's tiles: group-major fp16 x (+ones col) and chunk-local
    idx (transposed, fp16)."""
    xp = np.empty((Tpad * P, XC), dtype=np.float16)
    xp[:, :D] = padrow
    xp[:, D] = 1.0
    xp[:, D + 1] = 0.0
    idxoff = np.full((Tpad * P,), SENT, dtype=np.float16)
    base = 0
    for k in range(C):
        a, b2 = chunk_segs[k]
        m0, m1 = int(bounds[a]), int(bounds[b2])
        L = m1 - m0
        r0 = base * P
        xp[r0:r0 + L, :D] = x16[m0:m1]
        idxoff[r0:r0 + L] = (batch_idx[m0:m1] - a).astype(np.float16)
        base += Tc[k]
    ng = Tpad // G
    xg = np.ascontiguousarray(
        xp.reshape(ng, G, P, XC).transpose(0, 2, 1, 3).reshape(ng * P, G * XC))
    idxT = np.ascontiguousarray(idxoff.reshape(Tpad, P).T)
    return {"xg": xg, "idxT": idxT}


# ---------------------------------------------------------------- program
def _build_program(C, Tc):
    T = sum(Tc)
    Tpad = -(-T // G) * G
    NG = Tpad // G
    Alu = mybir.AluOpType
    Act = mybir.ActivationFunctionType
    W2 = 2 * SPAN

    chunk_of = {}
    first_t = {}
    last_t = {}
    base = 0
    for k in range(C):
        for j in range(Tc[k]):
            chunk_of[base + j] = k
        first_t[k] = base
        last_t[k] = base + Tc[k] - 1
        base += Tc[k]

    nc = bacc.Bacc("TRN2", target_bir_lowering=False, debug=False,
                   num_devices=NCORES)
    xg = nc.dram_tensor("xg", [NG * P, G * XC], f16, kind="ExternalInput").ap()
    idxT = nc.dram_tensor("idxT", [P, Tpad], f16, kind="ExternalInput").ap()
    wrep = nc.dram_tensor("wrep", [P, D], f16, kind="ExternalInput").ap()
    rowbd = nc.dram_tensor("rowbd", [P, SPAN], f16, kind="ExternalInput").ap()
    out = nc.dram_tensor("out", [P, (-(-C // 4)) * D], f32,
                         kind="ExternalOutput").ap()
    cc_in = nc.dram_tensor("cc_in", [1, 1], f32)
    cc_out = nc.dram_tensor("cc_out", [1, 1], f32, addr_space="Shared")
    groups = [list(range(NCORES))]

    with tile.TileContext(nc) as tc, ExitStack() as ctx:
        const = ctx.enter_context(tc.tile_pool(name="const", bufs=1))
        wrep_sb = const.tile([P, D], f16, tag="wrep")
        idxT_sb = const.tile([P, Tpad], f16, tag="idxT")
        rowb = const.tile([P, SPAN], f16, tag="rowb")
        zcols = const.tile([P, NG], f32, tag="zcols")
        lz = const.tile([P, 1], f32, tag="lz")
        lzr = const.tile([P, 1], f32, tag="lzr")
        gz = const.tile([1, 1], f32, tag="gz")
        ainv = const.tile([1, 1], f32, tag="ainv")
        alpha_col = const.tile([P, 1], f32, tag="alphacol")
        absb = const.tile([P, C * RC], f32, tag="absb")
        NB = -(-C // 4)
        a0p = const.tile([P, NB * RC], f32, tag="a0p")
        a1p = const.tile([P, NB * RC], f32, tag="a1p")

        nc.sync.dma_start(wrep_sb[:], wrep[:, :])
        nc.sync.dma_start(idxT_sb[:], idxT[:, :])
        nc.sync.dma_start(rowb[:], rowbd[:, :])

        xpool = ctx.enter_context(tc.tile_pool(name="xg", bufs=4))
        prodpool = ctx.enter_context(tc.tile_pool(name="prod", bufs=3))
        tpools = {w: ctx.enter_context(tc.tile_pool(name=f"t{w}", bufs=2))
                  for w in (128, 64, 32, 16, 8)}
        sgpool = ctx.enter_context(tc.tile_pool(name="sg", bufs=3))
        ugpool = ctx.enter_context(tc.tile_pool(name="ug", bufs=3))
        lpool = ctx.enter_context(tc.tile_pool(name="lhsT", bufs=4))
        psumpool = ctx.enter_context(
            tc.tile_pool(name="psum", bufs=3, space="PSUM"))
        ps = [None] * C

        for gi in range(NG):
            xg_sb = xpool.tile([P, G * XC], f16, tag="xg")
            nc.sync.dma_start(xg_sb[:], xg[gi * P:(gi + 1) * P, :])
            xv = xg_sb[:].rearrange("p (g c) -> p g c", g=G)

            prod = prodpool.tile([P, G * D], f16, tag="prod")
            pv = prod[:].rearrange("p (g c) -> p g c", g=G)
            nc.vector.tensor_tensor(
                out=pv, in0=xv[:, :, 0:D],
                in1=wrep_sb[:].unsqueeze(1).broadcast_to([P, G, D]),
                op=Alu.mult)
            cur = pv
            for w in (128, 64, 32, 16, 8):
                nt = tpools[w].tile([P, G * w], f16, tag=f"t{w}")
                nv = nt[:].rearrange("p (g c) -> p g c", g=G)
                nc.vector.tensor_tensor(out=nv, in0=cur[:, :, 0:w],
                                        in1=cur[:, :, w:2 * w], op=Alu.add)
                cur = nv
            sg = sgpool.tile([P, G], f32, tag="sg")
            nc.vector.tensor_reduce(out=sg[:], in_=cur,
                                    axis=mybir.AxisListType.X, op=Alu.add)
            ug = ugpool.tile([P, G], f16, tag="ug")
            nc.scalar.activation(ug[:], sg[:], Act.Exp,
                                 accum_out=zcols[:, gi:gi + 1])

            # one-hot build, transposed [P, 2*SPAN, G] (on the idle gpsimd)
            lhsTg = lpool.tile([P, W2 * G], f16, tag="lhsT")
            lv = lhsTg[:].rearrange("p (j g) -> p j g", g=G)
            nc.vector.tensor_tensor(
                out=lv[:, 0:SPAN, :],
                in0=rowb[:].unsqueeze(2).broadcast_to([P, SPAN, G]),
                in1=idxT_sb[:, gi * G:(gi + 1) * G].unsqueeze(1)
                .broadcast_to([P, SPAN, G]),
                op=Alu.is_equal)
            nc.vector.tensor_tensor(
                out=lv[:, SPAN:W2, :], in0=lv[:, 0:SPAN, :],
                in1=ug[:].unsqueeze(1).broadcast_to([P, SPAN, G]),
                op=Alu.mult)

            for g in range(G):
                t = gi * G + g
                if t not in chunk_of:
                    continue
                k = chunk_of[t]
                if t == first_t[k]:
                    ps[k] = psumpool.tile([W2, RC], f32, tag="ps", name="pschunk")
                nc.tensor.matmul(ps[k][:], lhsT=lv[:, :, g],
                                 rhs=xv[:, g, 0:RC], start=(t == first_t[k]),
                                 stop=(t == last_t[k]))
                if t == last_t[k]:
                    nc.scalar.copy(absb[0:W2, k * RC:(k + 1) * RC], ps[k][:])

        # repack chunks across all 128 partitions: chunk k -> partition
        # block (k%4)*SPAN, column block k//4 (SBUF->SBUF DMA remaps)
        av = absb[:].rearrange("p (k c) -> p k c", k=C)
        for a in range(4):
            nb_a = len(range(a, C, 4))
            nc.sync.dma_start(
                a0p[a * SPAN:(a + 1) * SPAN, 0:nb_a * RC]
                .rearrange("p (k c) -> p k c", k=nb_a),
                av[0:SPAN, a::4, :])
            nc.sync.dma_start(
                a1p[a * SPAN:(a + 1) * SPAN, 0:nb_a * RC]
                .rearrange("p (k c) -> p k c", k=nb_a),
                av[SPAN:W2, a::4, :])

        # ---- global softmax denominator Z0 and alpha = 1/Z0 ----
        nc.vector.tensor_reduce(out=lz[:], in_=zcols[:],
                                axis=mybir.AxisListType.X, op=Alu.add)
        nc.gpsimd.partition_all_reduce(lzr[:], lz[:], channels=P,
                                       reduce_op=bass_isa.ReduceOp.add)
        nc.sync.dma_start(cc_in[:, :], lzr[0:1, 0:1])
        nc.gpsimd.collective_compute(
            "AllReduce", Alu.add, replica_groups=groups,
            ins=[cc_in[:, :]], outs=[cc_out[:, :]])
        nc.sync.dma_start(gz[:], cc_out[:, :])
        nc.vector.reciprocal(ainv[:], gz[:])
        nc.gpsimd.partition_broadcast(alpha_col[:], ainv[:])

        # ---- grouped combine: (A0 + a*A1) / (c0 + a*c1) for all chunks ----
        tailpool = ctx.enter_context(tc.tile_pool(name="tail", bufs=1))
        nc.vector.tensor_scalar(out=a1p[:], in0=a1p[:],
                                scalar1=alpha_col[:, 0:1], scalar2=None,
                                op0=Alu.mult)
        nc.vector.tensor_tensor(out=a1p[:], in0=a1p[:], in1=a0p[:],
                                op=Alu.add)
        nv = a1p[:].rearrange("p (k c) -> p k c", k=NB)
        den = tailpool.tile([P, NB], f32, tag="den")
        nc.vector.tensor_scalar_max(den[:], nv[:, :, D], 0.5)
        rec = tailpool.tile([P, NB], f32, tag="rec")
        nc.vector.reciprocal(rec[:], den[:])
        ov = a0p[:].rearrange("p (k c) -> p k c", k=NB)[:, :, 0:D]
        nc.vector.tensor_tensor(
            out=ov, in0=nv[:, :, 0:D],
            in1=rec[:].unsqueeze(2).broadcast_to([P, NB, D]), op=Alu.mult)
        nc.sync.dma_start(out.rearrange("p (k c) -> p k c", k=NB), ov)

    nc.compile()
    return nc


def _get_program(C, Tc):
    key = (C, tuple(Tc))
    if key not in _prog_cache:
        _prog_cache[key] = _build_program(C, Tc)
    return _prog_cache[key]


# ---------------------------------------------------------------- entry
def kernel(x, batch_idx, W, b, num_segments):
    x = np.asarray(x, dtype=np.float32)
    batch_idx = np.asarray(batch_idx)
    W = np.asarray(W, dtype=np.float32)
    assert int(num_segments) == NSEG and x.shape[1] == D

    core_seg, chunk_seg, C, Tc, bounds = _plan(batch_idx)
    T = sum(Tc)
    Tpad = -(-T // G) * G
    nc = _get_program(C, Tc)

    x16 = x.astype(np.float16)
    w16 = W[:, 0].astype(np.float16)
    wrep = np.ascontiguousarray(np.broadcast_to(w16, (P, D)))
    rowbd = np.ascontiguousarray(
        np.broadcast_to(np.arange(SPAN, dtype=np.float16), (P, SPAN)))
    # padding rows: x chosen so s = -5*sum|W| => exp(s) ~ 0 (keeps Z exact)
    padrow = (-5.0 * np.sign(w16)).astype(np.float16)

    in_maps = []
    for c in range(NCORES):
        m = _build_core_inputs(x16, batch_idx, padrow, chunk_seg[c], bounds,
                               C, Tc, Tpad)
        m["wrep"] = wrep
        m["rowbd"] = rowbd
        in_maps.append(m)

    global LAST_EXEC_NS
    res = bass_utils.run_bass_kernel_spmd(
        nc, in_maps, core_ids=list(range(NCORES)), trace=TRACE)
    if res.exec_time_ns is not None:
        LAST_EXEC_NS = res.exec_time_ns

    full = np.zeros((NSEG, D), dtype=np.float32)
    for c in range(NCORES):
        oc = res.results[c]["out"]
        for k in range(C):
            a, b2 = chunk_seg[c][k]
            p0 = (k % 4) * SPAN
            bblk = k // 4
            full[a:b2] = oc[p0:p0 + (b2 - a), bblk * D:(bblk + 1) * D]
    return full


# revision 4
# speedup vs baseline: 1.1117x; 1.1070x over previous
"""AttentionPooling (segment softmax-pool) Trainium2 kernel, 8-way data parallel.

Math: s = x@W + b; g = softmax(s) over all N; pooled[seg] = softmax-weighted
sum of x with weights softmax_seg(g).  The bias b cancels exactly in g, and
the per-segment max-shift cancels in the final softmax, so
  w_i  =  exp(g_i) / sum_seg exp(g_j),   g_i = exp(s_i) / Z0,  Z0 = sum exp(s).
Since g_i is tiny here, exp(g_i) = 1 + g_i to ~1e-9: the pooled output is
  (A0 + a*A1) / (c0 + a*c1),  a = 1/Z0,
  A0 = segsum(x), A1 = segsum(u*x), c0 = counts, c1 = segsum(u), u = exp(s).
All four accumulate in ONE matmul per 128-node tile via a stacked one-hot
lhsT: columns 0..SPAN-1 hold onehot(seg), columns SPAN..2*SPAN-1 onehot*u.
A single 1-scalar AllReduce produces Z0; a grouped combine finishes.

Single pass over x (fp16-packed on host), G tiles per DMA.  s is computed
on-device: packed multiply by W then a binary-tree reduction on the vector
engine (all operands packed-last for the 4x DVE mode); exp per group on the
scalar engine.  One-hot builds use a transposed [P, 2*SPAN, G] layout so
every operand keeps a packed last dim; the matmul reads strided lhsT slices.
"""

from contextlib import ExitStack

import numpy as np

import concourse.bass_isa as bass_isa
import concourse.tile as tile
from concourse import bacc, mybir, bass_utils

P = 128
D = 256
XC = D + 2          # x, ones column, even-stride pad column
RC = D + 1          # matmul rhs columns (x + ones)
NCORES = 8
NSEG = 4096
SPAN = 32           # max segments per chunk (stacked one-hot: 2*SPAN lhsT cols)
G = 32              # tiles per DMA/compute group
SENT = 500.0        # idx value for padding rows; never matches 0..SPAN-1

f16 = mybir.dt.float16
f32 = mybir.dt.float32

_prog_cache = {}

TRACE = False
LAST_EXEC_NS = None


# ---------------------------------------------------------------- host plan
def _snap(bounds, tgt, lo, hi):
    s = int(np.searchsorted(bounds, tgt))
    if s > 0 and abs(int(bounds[s - 1]) - tgt) < abs(int(bounds[s]) - tgt):
        s -= 1
    return max(lo, min(s, hi))


def _plan(batch_idx):
    N = batch_idx.shape[0]
    counts = np.bincount(batch_idx, minlength=NSEG)
    bounds = np.concatenate([[0], np.cumsum(counts)]).astype(np.int64)

    core_seg = [0]
    for c in range(1, NCORES):
        s = _snap(bounds, N * c // NCORES, core_seg[-1] + 1, NSEG - (NCORES - c))
        core_seg.append(s)
    core_seg.append(NSEG)

    C = max(-(-(core_seg[c + 1] - core_seg[c]) // SPAN) for c in range(NCORES))

    chunk_seg = []
    for c in range(NCORES):
        s0c, s1c = core_seg[c], core_seg[c + 1]
        n0c, n1c = int(bounds[s0c]), int(bounds[s1c])
        ks = [s0c]
        for k in range(1, C):
            lo = max(ks[-1] + 1, s1c - SPAN * (C - k))
            hi = min(ks[-1] + SPAN, s1c - (C - k) + 1)
            s = _snap(bounds, n0c + (n1c - n0c) * k // C, lo, hi)
            ks.append(s)
        ks.append(s1c)
        segs = list(zip(ks[:-1], ks[1:]))
        for a, b2 in segs:
            assert 0 <= b2 - a <= SPAN, f"chunk with {b2 - a} segments"
        chunk_seg.append(segs)

    Tc = []
    for k in range(C):
        mx = 1
        for c in range(NCORES):
            a, b2 = chunk_seg[c][k]
            mx = max(mx, -(-int(bounds[b2] - bounds[a]) // P))
        Tc.append(mx)
    return core_seg, chunk_seg, C, Tc, bounds


def _build_core_inputs(x16, batch_idx, padrow, chunk_segs, bounds, C, Tc, Tpad):
    """Pack one core's tiles: group-major fp16 x (+ones col) and chunk-local
    idx (transposed, fp16)."""
    xp = np.empty((Tpad * P, XC), dtype=np.float16)
    xp[:, :D] = padrow
    xp[:, D] = 1.0
    xp[:, D + 1] = 0.0
    idxoff = np.full((Tpad * P,), SENT, dtype=np.float16)
    base = 0
    for k in range(C):
        a, b2 = chunk_segs[k]
        m0, m1 = int(bounds[a]), int(bounds[b2])
        L = m1 - m0
        r0 = base * P
        xp[r0:r0 + L, :D] = x16[m0:m1]
        idxoff[r0:r0 + L] = (batch_idx[m0:m1] - a).astype(np.float16)
        base += Tc[k]
    ng = Tpad // G
    xg = np.ascontiguousarray(
        xp.reshape(ng, G, P, XC).transpose(0, 2, 1, 3).reshape(ng * P, G * XC))
    idxT = np.ascontiguousarray(idxoff.reshape(Tpad, P).T)
    return {"xg": xg, "idxT": idxT}


# ---------------------------------------------------------------- program
def _build_program(C, Tc):
    T = sum(Tc)
    Tpad = -(-T // G) * G
    NG = Tpad // G
    Alu = mybir.AluOpType
    Act = mybir.ActivationFunctionType
    W2 = 2 * SPAN

    chunk_of = {}
    first_t = {}
    last_t = {}
    base = 0
    for k in range(C):
        for j in range(Tc[k]):
            chunk_of[base + j] = k
        first_t[k] = base
        last_t[k] = base + Tc[k] - 1
        base += Tc[k]

    nc = bacc.Bacc("TRN2", target_bir_lowering=False, debug=False,
                   num_devices=NCORES)
    xg = nc.dram_tensor("xg", [NG * P, G * XC], f16, kind="ExternalInput").ap()
    idxT = nc.dram_tensor("idxT", [P, Tpad], f16, kind="ExternalInput").ap()
    wrep = nc.dram_tensor("wrep", [P, D], f16, kind="ExternalInput").ap()
    rowbd = nc.dram_tensor("rowbd", [P, SPAN * G], f16,
                           kind="ExternalInput").ap()
    out = nc.dram_tensor("out", [P, (-(-C // 4)) * D], f32,
                         kind="ExternalOutput").ap()
    cc_in = nc.dram_tensor("cc_in", [1, 1], f32)
    cc_out = nc.dram_tensor("cc_out", [1, 1], f32, addr_space="Shared")
    groups = [list(range(NCORES))]

    with tile.TileContext(nc) as tc, ExitStack() as ctx:
        const = ctx.enter_context(tc.tile_pool(name="const", bufs=1))
        wrep_sb = const.tile([P, D], f16, tag="wrep")
        idxT_sb = const.tile([P, Tpad], f16, tag="idxT")
        rowb = const.tile([P, SPAN * G], f16, tag="rowb")
        zcols = const.tile([P, NG], f32, tag="zcols")
        lz = const.tile([P, 1], f32, tag="lz")
        lzr = const.tile([P, 1], f32, tag="lzr")
        gz = const.tile([1, 1], f32, tag="gz")
        ainv = const.tile([1, 1], f32, tag="ainv")
        alpha_col = const.tile([P, 1], f32, tag="alphacol")
        absb = const.tile([P, C * RC], f32, tag="absb")
        NB = -(-C // 4)
        a0p = const.tile([P, NB * RC], f32, tag="a0p")
        a1p = const.tile([P, NB * RC], f32, tag="a1p")

        nc.sync.dma_start(wrep_sb[:], wrep[:, :])
        nc.sync.dma_start(idxT_sb[:], idxT[:, :])
        nc.sync.dma_start(rowb[:], rowbd[:, :])

        xpool = ctx.enter_context(tc.tile_pool(name="xg", bufs=4))
        prodpool = ctx.enter_context(tc.tile_pool(name="prod", bufs=3))
        tpools = {w: ctx.enter_context(tc.tile_pool(name=f"t{w}", bufs=2))
                  for w in (128, 64, 32, 16, 8)}
        sgpool = ctx.enter_context(tc.tile_pool(name="sg", bufs=4))
        ugpool = ctx.enter_context(tc.tile_pool(name="ug", bufs=4))
        lpool = ctx.enter_context(tc.tile_pool(name="lhsT", bufs=4))
        psumpool = ctx.enter_context(
            tc.tile_pool(name="psum", bufs=4, space="PSUM"))
        ps = [None] * C

        for gi in range(NG):
            xg_sb = xpool.tile([P, G * XC], f16, tag="xg")
            nc.sync.dma_start(xg_sb[:], xg[gi * P:(gi + 1) * P, :])
            xv = xg_sb[:].rearrange("p (g c) -> p g c", g=G)

            prod = prodpool.tile([P, G * D], f16, tag="prod")
            pv = prod[:].rearrange("p (g c) -> p g c", g=G)
            nc.vector.tensor_tensor(
                out=pv, in0=xv[:, :, 0:D],
                in1=wrep_sb[:].unsqueeze(1).broadcast_to([P, G, D]),
                op=Alu.mult)
            cur = pv
            for w in (128, 64, 32, 16, 8):
                nt = tpools[w].tile([P, G * w], f16, tag=f"t{w}")
                nv = nt[:].rearrange("p (g c) -> p g c", g=G)
                nc.vector.tensor_tensor(out=nv, in0=cur[:, :, 0:w],
                                        in1=cur[:, :, w:2 * w], op=Alu.add)
                cur = nv
            sg = sgpool.tile([P, G], f32, tag="sg")
            nc.vector.tensor_reduce(out=sg[:], in_=cur,
                                    axis=mybir.AxisListType.X, op=Alu.add)
            ug = ugpool.tile([P, G], f16, tag="ug")
            nc.scalar.activation(ug[:], sg[:], Act.Exp,
                                 accum_out=zcols[:, gi:gi + 1])

            # one-hot build, transposed [P, 2*SPAN, G] (on the idle gpsimd)
            lhsTg = lpool.tile([P, W2 * G], f16, tag="lhsT")
            lv = lhsTg[:].rearrange("p (j g) -> p j g", g=G)
            nc.vector.tensor_tensor(
                out=lv[:, 0:SPAN, :],
                in0=rowb[:].rearrange("p (j g) -> p j g", g=G),
                in1=idxT_sb[:, gi * G:(gi + 1) * G].unsqueeze(1)
                .broadcast_to([P, SPAN, G]),
                op=Alu.is_equal)
            nc.vector.tensor_tensor(
                out=lv[:, SPAN:W2, :], in0=lv[:, 0:SPAN, :],
                in1=ug[:].unsqueeze(1).broadcast_to([P, SPAN, G]),
                op=Alu.mult)

            for g in range(G):
                t = gi * G + g
                if t not in chunk_of:
                    continue
                k = chunk_of[t]
                if t == first_t[k]:
                    ps[k] = psumpool.tile([W2, RC], f32, tag="ps", name="pschunk")
                nc.tensor.matmul(ps[k][:], lhsT=lv[:, :, g],
                                 rhs=xv[:, g, 0:RC], start=(t == first_t[k]),
                                 stop=(t == last_t[k]))
                if t == last_t[k]:
                    nc.scalar.copy(absb[0:W2, k * RC:(k + 1) * RC], ps[k][:])

        # repack chunks across all 128 partitions: chunk k -> partition
        # block (k%4)*SPAN, column block k//4 (SBUF->SBUF DMA remaps)
        av = absb[:].rearrange("p (k c) -> p k c", k=C)
        for a in range(4):
            nb_a = len(range(a, C, 4))
            nc.sync.dma_start(
                a0p[a * SPAN:(a + 1) * SPAN, 0:nb_a * RC]
                .rearrange("p (k c) -> p k c", k=nb_a),
                av[0:SPAN, a::4, :])
            nc.sync.dma_start(
                a1p[a * SPAN:(a + 1) * SPAN, 0:nb_a * RC]
                .rearrange("p (k c) -> p k c", k=nb_a),
                av[SPAN:W2, a::4, :])

        # ---- global softmax denominator Z0 and alpha = 1/Z0 ----
        nc.vector.tensor_reduce(out=lz[:], in_=zcols[:],
                                axis=mybir.AxisListType.X, op=Alu.add)
        nc.gpsimd.partition_all_reduce(lzr[:], lz[:], channels=P,
                                       reduce_op=bass_isa.ReduceOp.add)
        nc.sync.dma_start(cc_in[:, :], lzr[0:1, 0:1])
        nc.gpsimd.collective_compute(
            "AllReduce", Alu.add, replica_groups=groups,
            ins=[cc_in[:, :]], outs=[cc_out[:, :]])
        nc.sync.dma_start(gz[:], cc_out[:, :])
        nc.vector.reciprocal(ainv[:], gz[:])
        nc.gpsimd.partition_broadcast(alpha_col[:], ainv[:])

        # ---- grouped combine: (A0 + a*A1) / (c0 + a*c1) for all chunks ----
        tailpool = ctx.enter_context(tc.tile_pool(name="tail", bufs=1))
        nc.vector.tensor_scalar(out=a1p[:], in0=a1p[:],
                                scalar1=alpha_col[:, 0:1], scalar2=None,
                                op0=Alu.mult)
        nc.vector.tensor_tensor(out=a1p[:], in0=a1p[:], in1=a0p[:],
                                op=Alu.add)
        nv = a1p[:].rearrange("p (k c) -> p k c", k=NB)
        den = tailpool.tile([P, NB], f32, tag="den")
        nc.vector.tensor_scalar_max(den[:], nv[:, :, D], 0.5)
        rec = tailpool.tile([P, NB], f32, tag="rec")
        nc.vector.reciprocal(rec[:], den[:])
        ov = a0p[:].rearrange("p (k c) -> p k c", k=NB)[:, :, 0:D]
        nc.vector.tensor_tensor(
            out=ov, in0=nv[:, :, 0:D],
            in1=rec[:].unsqueeze(2).broadcast_to([P, NB, D]), op=Alu.mult)
        nc.sync.dma_start(out.rearrange("p (k c) -> p k c", k=NB), ov)

    nc.compile()
    return nc


def _get_program(C, Tc):
    key = (C, tuple(Tc))
    if key not in _prog_cache:
        _prog_cache[key] = _build_program(C, Tc)
    return _prog_cache[key]


# ---------------------------------------------------------------- entry
def kernel(x, batch_idx, W, b, num_segments):
    x = np.asarray(x, dtype=np.float32)
    batch_idx = np.asarray(batch_idx)
    W = np.asarray(W, dtype=np.float32)
    assert int(num_segments) == NSEG and x.shape[1] == D

    core_seg, chunk_seg, C, Tc, bounds = _plan(batch_idx)
    T = sum(Tc)
    Tpad = -(-T // G) * G
    nc = _get_program(C, Tc)

    x16 = x.astype(np.float16)
    w16 = W[:, 0].astype(np.float16)
    wrep = np.ascontiguousarray(np.broadcast_to(w16, (P, D)))
    rowbd = np.ascontiguousarray(np.broadcast_to(
        np.repeat(np.arange(SPAN, dtype=np.float16), G), (P, SPAN * G)))
    # padding rows: x chosen so s = -5*sum|W| => exp(s) ~ 0 (keeps Z exact)
    padrow = (-5.0 * np.sign(w16)).astype(np.float16)

    in_maps = []
    for c in range(NCORES):
        m = _build_core_inputs(x16, batch_idx, padrow, chunk_seg[c], bounds,
                               C, Tc, Tpad)
        m["wrep"] = wrep
        m["rowbd"] = rowbd
        in_maps.append(m)

    global LAST_EXEC_NS
    res = bass_utils.run_bass_kernel_spmd(
        nc, in_maps, core_ids=list(range(NCORES)), trace=TRACE)
    if res.exec_time_ns is not None:
        LAST_EXEC_NS = res.exec_time_ns

    full = np.zeros((NSEG, D), dtype=np.float32)
    for c in range(NCORES):
        oc = res.results[c]["out"]
        for k in range(C):
            a, b2 = chunk_seg[c][k]
            p0 = (k % 4) * SPAN
            bblk = k // 4
            full[a:b2] = oc[p0:p0 + (b2 - a), bblk * D:(bblk + 1) * D]
    return full
